# revision 5
# baseline (speedup 1.0000x reference)
"""Trainium2 Bass kernel for ConformalGQA, v2.

Math identical to reference modulo softmax shift invariance: the -0.5|q|^2
term in the scores is constant over the softmax (key) axis, so it is dropped
entirely. Scores become (q.k - 0.5|k|^2)/8, bounded above by |q|^2/16 ~ 6, so
exp never overflows fp32 and needs no max pass. The -0.5|k|^2/8 term rides
the per-partition bias of the Exp activation.

Sharding: 8-way tensor-parallel over heads (core c: Q heads 4c..4c+3, KV
head c). Each core emits a full (4096, 2048) bf16 partial; host sums.

Per core, per batch (t=1024):
 - xT chunks DMA'd bf16; Wq/Wk/Wv column shards projected with weights
   stationary into PSUM f32 chunks [128, 512].
 - RoPE: PSUM chunk evicted to SBUF f32 (Act), cos-mul + signed-sin
   shifted-muls (shift = +-32 partitions; muls on GPSIMD, add on DVE),
   emitted as bf16 qhat/khat. khat duplicated to partitions 64:128 so both
   heads of a pair run S-matmuls via tile_position (0,0)/(64,0).
 - S^T computed per (head, kc) into [128, <=512] PSUM chunks with k on
   partitions (bf16 matmuls run 1 cyc/row at any free size); the diagonal
   block is zeroed post-exp by a triangular mask (DVE/Pool alternating).
 - P^T = Exp(S^T/8 + bias) -> bf16, bias = -0.0625|k|^2.
 - PV: yhat[65, q] += [V|1].T @ P^T accumulated over kc into two
   single-bank [65, 512] PSUM halves; row 64 is the softmax denominator.
   The low half completes at kc=3 and normalizes/frees early.
 - normalize: DVE reciprocal row, GPSIMD partition_broadcast, fused
   mul (PSUM x SBUF) -> ytn bf16.
 - out proj: ytn as lhsT against Wo row-shard, PSUM chunks evicted bf16
   (alternating DVE/Act) and DMA'd out per 128-token row block.
 - Whole thing software-pipelined: proj/rope of batch b+1 is issued at
   higher scheduler priority than attention/outproj of batch b.
 - NOTE: InstDmaTransposeAnt produced NaN on hardware under this runtime
   (CoreSim was clean) - V is transposed on the PE instead.
"""

import sys

for _p in ("/opt/trn_rl_repo",):
    if _p not in sys.path:
        sys.path.insert(0, _p)

import numpy as np
import ml_dtypes
from contextlib import ExitStack

import concourse.bass as bass
import concourse.mybir as mybir
import concourse.tile as tile
from concourse import bacc
from concourse.bass_utils import run_bass_kernel_spmd

F32R = mybir.dt.float32r
F32 = mybir.dt.float32
BF16 = mybir.dt.bfloat16
AF = mybir.ActivationFunctionType
BF = ml_dtypes.bfloat16

B, T, D, KV = 4, 1024, 2048, 512
H, HKV, HD = 32, 8, 64
P = 128
NCORES = 8
HPC = H // NCORES          # 4 q heads per core
DOUT = HPC * HD            # 256 q-proj cols per core
NDC = D // P               # 16 contraction chunks
NTC = T // P               # 8 token chunks per batch
ROPE_BASE = 10000.0

_COMPILED = {}


def _chunks_for(kc):
    """Natural S/PV q-chunks for key block kc (bf16: any width is full
    rate). Chunks never straddle the 512 boundary (PSUM half split)."""
    q0 = kc * P
    out = []
    c0 = q0
    while c0 < T:
        c1 = min(T, 512 if c0 < 512 else T)
        out.append((c0, c1))
        c0 = c1
    return q0, out


def _build_nc():
    nc = bacc.Bacc("TRN2", target_bir_lowering=False, debug=False,
                   num_devices=NCORES)

    xT = nc.dram_tensor("xT", [D, B * T], BF16, kind="ExternalInput")
    wq = nc.dram_tensor("wq", [P, NDC, DOUT], BF16, kind="ExternalInput")
    wkv = nc.dram_tensor("wkv", [P, NDC, 2 * HD], BF16, kind="ExternalInput")
    wo = nc.dram_tensor("wo", [P, 2, D], BF16, kind="ExternalInput")
    cc = nc.dram_tensor("cc", [P, T], F32, kind="ExternalInput")
    ss = nc.dram_tensor("ss", [P, T], F32, kind="ExternalInput")
    tri2 = nc.dram_tensor("tri2", [P, 2, P], BF16, kind="ExternalInput")
    o64 = nc.dram_tensor("o64", [64, 2], F32R, kind="ExternalInput")
    o1x64 = nc.dram_tensor("o1x64", [1, 64], F32R, kind="ExternalInput")
    idb = nc.dram_tensor("idb", [64, 64], BF16, kind="ExternalInput")
    out = nc.dram_tensor("out", [B * T, D], BF16, kind="ExternalOutput")

    with tile.TileContext(nc) as tc:
        with ExitStack() as ctx:
            cpool = ctx.enter_context(tc.tile_pool(name="consts", bufs=1))
            wpool = ctx.enter_context(tc.tile_pool(name="weights", bufs=1))
            xpool = ctx.enter_context(tc.tile_pool(name="x", bufs=8))
            spool = ctx.enter_context(tc.tile_pool(name="stage", bufs=4))
            qpool = ctx.enter_context(tc.tile_pool(name="qk", bufs=2))
            vpool = ctx.enter_context(tc.tile_pool(name="v", bufs=2))
            fpool = ctx.enter_context(tc.tile_pool(name="pt", bufs=3))
            npool = ctx.enter_context(tc.tile_pool(name="norm", bufs=4))
            ypool = ctx.enter_context(tc.tile_pool(name="ytn", bufs=2))
            opool = ctx.enter_context(tc.tile_pool(name="ostg", bufs=3))
            psy = ctx.enter_context(tc.tile_pool(name="psy", bufs=2, space="PSUM"))
            pss = ctx.enter_context(tc.tile_pool(name="pss", bufs=4, space="PSUM"))
            psm = ctx.enter_context(tc.tile_pool(name="psm", bufs=2, space="PSUM"))

            # ---- early consts (needed by first projections/rope) ----
            t_wkv = wpool.tile([P, NDC, 2 * HD], BF16)
            nc.sync.dma_start(t_wkv[:], wkv.ap())
            t_wq = wpool.tile([P, NDC, DOUT], BF16)
            t_cc = cpool.tile([P, T], F32)
            t_ss = cpool.tile([P, T], F32)

            xT3 = xT.ap().rearrange("(c p) t -> p c t", p=P)  # [128, 16, 4096]

            def late_consts():
                t_tri2 = cpool.tile([P, 2, P], BF16)
                nc.sync.dma_start(t_tri2[:], tri2.ap())
                t_o64 = cpool.tile([64, 2], F32R)
                nc.sync.dma_start(t_o64[:], o64.ap())
                t_o1x64 = cpool.tile([1, 64], F32R)
                nc.sync.dma_start(t_o1x64[:], o1x64.ap())
                t_idb = cpool.tile([64, 64], BF16)
                nc.sync.dma_start(t_idb[:], idb.ap())
                t_wo = wpool.tile([P, 2, D], BF16)
                nc.sync.dma_start(t_wo[:], wo.ap())
                return t_tri2, t_o64, t_wo, t_o1x64, t_idb

            lc = None

            def rope_half(pj, dst, rows, half, sign_dup):
                """Evict PSUM proj chunk, rope it, write bf16 into dst."""
                c0 = half * 512
                sb = spool.tile([P, 512], F32, tag="qsb")
                nc.vector.tensor_copy(sb[0:rows, :], pj[0:rows, :])
                t1 = spool.tile([P, 512], F32, tag="t1")
                nc.vector.tensor_mul(
                    t1[0:rows, :], sb[0:rows, :], t_cc[0:rows, c0:c0 + 512])
                t2 = spool.tile([P, 512], F32, tag="t2")
                for bp2 in range(0, rows, 64):
                    nc.gpsimd.tensor_mul(
                        t2[bp2:bp2 + 32, :], sb[bp2 + 32:bp2 + 64, :],
                        t_ss[bp2 + 32:bp2 + 64, c0:c0 + 512])
                    nc.gpsimd.tensor_mul(
                        t2[bp2 + 32:bp2 + 64, :], sb[bp2:bp2 + 32, :],
                        t_ss[bp2:bp2 + 32, c0:c0 + 512])
                nc.vector.tensor_add(
                    dst[0:rows, c0:c0 + 512], t1[0:rows, :], t2[0:rows, :])
                if sign_dup:
                    nc.vector.tensor_copy(
                        dst[64:128, c0:c0 + 512], dst[0:64, c0:c0 + 512])

            def proj_rope_stage(b):
                """Load xT for batch b, project Q/K/V, rope, prep vh/kb."""
                nonlocal lc
                tok0 = b * T
                xts = []
                for qtr in range(4):
                    xt = xpool.tile([P, 4, T], BF16, tag="xt",
                                    name=f"xt_{b}_{qtr}")
                    if b == 0:
                        # fine-grained loads so batch-0 projections start
                        # as soon as the first contraction chunk lands
                        for i in range(4):
                            nc.sync.dma_start(
                                xt[:, i, :],
                                xT3[:, qtr * 4 + i, tok0:tok0 + T])
                        if qtr == 0:
                            nc.sync.dma_start(t_wq[:], wq.ap())
                        if qtr == 2:
                            nc.sync.dma_start(t_cc[:], cc.ap())
                            nc.sync.dma_start(t_ss[:], ss.ap())
                    else:
                        nc.sync.dma_start(
                            xt[:], xT3[:, qtr * 4:(qtr + 1) * 4, tok0:tok0 + T])
                    xts.append(xt)
                if b == 0:
                    lc = late_consts()

                def xsrc(dc):
                    return xts[dc // 4][:, dc % 4, :]

                qh = [qpool.tile([P, T], BF16, tag="qh", bufs=4,
                                 name=f"qh_{b}_{i}") for i in range(2)]
                kh = qpool.tile([P, T], BF16, tag="kh", name=f"kh_{b}")
                k2 = qpool.tile([64, T], F32R, tag="k2", name=f"k2_{b}")
                vt = vpool.tile([64, T], BF16, tag="vt", name=f"vt_{b}")

                # interleave kv and q-pair0 chunks so both pj slots
                # stream against arriving xT chunks; then q-pair1.
                def kv_chunk(half):
                    pj = psm.tile([P, 512], F32, tag="pj",
                                  name=f"kvpj_{b}_{half}")
                    for dc in range(NDC):
                        nc.tensor.matmul(
                            pj[:], t_wkv[:, dc, :],
                            xsrc(dc)[:, half * 512:(half + 1) * 512],
                            start=(dc == 0), stop=(dc == NDC - 1))
                    # kv proj out rows 0:64 = K dims, 64:128 = V dims.
                    nc.vector.tensor_copy(vt[:, half * 512:(half + 1) * 512],
                                          pj[64:128, :])
                    rope_half(pj, kh, 64, half, sign_dup=True)

                def q_chunk(pairi, half):
                    pj = psm.tile([P, 512], F32, tag="pj",
                                  name=f"qpj_{b}_{pairi}_{half}")
                    for dc in range(NDC):
                        nc.tensor.matmul(
                            pj[:],
                            t_wq[:, dc, pairi * P:(pairi + 1) * P],
                            xsrc(dc)[:, half * 512:(half + 1) * 512],
                            start=(dc == 0), stop=(dc == NDC - 1))
                    rope_half(pj, qh[pairi], 128, half, sign_dup=False)

                kv_chunk(0)
                q_chunk(0, 0)
                kv_chunk(1)
                q_chunk(0, 1)
                q_chunk(1, 0)
                q_chunk(1, 1)

                # |k|^2 -> per-partition bias  (transposed via PE)
                t_o64, t_idb = lc[1], lc[4]
                nc.scalar.activation(k2[:], kh[0:64, :], AF.Square)
                nsq = psm.tile([P, 512], F32, tag="pj", name=f"nsq_{b}")
                for kc in range(NTC):
                    nc.tensor.matmul(
                        nsq[:, 2 * kc:2 * kc + 2], k2[:, kc * P:(kc + 1) * P],
                        t_o64[:], start=True, stop=True)
                kb = qpool.tile([P, NTC], F32, tag="kb", name=f"kb_{b}")
                nc.vector.tensor_scalar_mul(
                    kb[:],
                    nsq[:, 0:2 * NTC]
                    .rearrange("p (c two) -> p c two", two=2)[:, :, 0],
                    -0.0625)

                # V transposed into [token, hd | 1] layout via PE transpose
                vh = vpool.tile([P, NTC, HD + 1], BF16, tag="vh",
                                name=f"vh_{b}")
                nc.vector.memset(vh[:, :, HD:HD + 1], 1.0)
                for tcn in range(NTC):
                    tp = pss.tile([P, 64], BF16, tag="stp", name=f"tp_{b}_{tcn}")
                    nc.tensor.transpose(
                        tp[:], vt[:, tcn * P:(tcn + 1) * P], t_idb[:])
                    nc.scalar.copy(vh[:, tcn, 0:HD], tp[:])
                return dict(b=b, qh=qh, kh=kh, kb=kb, vh=vh)

            def attn_out_stage(st):
                b, qh, kh, kb, vh = st["b"], st["qh"], st["kh"], st["kb"], st["vh"]
                tok0 = b * T
                t_tri2, t_o64, t_wo, t_o1x64, t_idb = lc
                ytn = [ypool.tile([P, T], BF16, tag="ytn", bufs=4,
                                  name=f"ytn_{b}_{i}") for i in range(2)]
                def normalize_half(yh_half, hq, pairi, bp, tag):
                    """One half of softmax-normalize as soon as its PV
                    contributions are complete; frees the yh slot early."""
                    rsb = npool.tile([1, 512], F32R, tag="rsb")
                    with nc.allow_low_precision(reason="recip row"):
                        nc.vector.reciprocal(rsb[:], yh_half[64:65, :])
                    rbc = npool.tile([64, 512], F32R, tag="rbc")
                    nc.gpsimd.partition_broadcast(rbc[:], rsb[:])
                    nc.vector.tensor_mul(
                        ytn[pairi][bp:bp + 64, hq * 512:(hq + 1) * 512],
                        yh_half[0:64, :], rbc[:])

                for h in range(HPC):
                    pairi, bp = h // 2, 64 * (h % 2)
                    yhA = psy.tile([65, 512], F32, tag="yh",
                                   name=f"yhA_{b}_{h}")
                    yhB = psy.tile([65, 512], F32, tag="yh",
                                   name=f"yhB_{b}_{h}")
                    for kc in range(NTC):
                        q0 = kc * P
                        _, chs = _chunks_for(kc)
                        pt = fpool.tile([P, T], BF16, tag="pt", bufs=6)
                        for (c0, c1) in chs:
                            stp = pss.tile([P, 512], F32, tag="stp")
                            nc.tensor.matmul(
                                stp[:, 0:c1 - c0],
                                kh[bp:bp + 64, kc * P:(kc + 1) * P],
                                qh[pairi][bp:bp + 64, c0:c1],
                                start=True, stop=True,
                                tile_position=(bp, 0))
                            nc.scalar.activation(
                                pt[:, c0:c1], stp[:, 0:c1 - c0], AF.Exp,
                                bias=kb[:, kc:kc + 1], scale=0.125)
                        # causal mask on the diagonal block
                        meng = nc.vector if kc % 2 == 0 else nc.gpsimd
                        meng.tensor_mul(
                            pt[:, q0:q0 + P], pt[:, q0:q0 + P],
                            t_tri2[:, 0, :])
                        for (c0, c1) in chs:
                            half = yhA if c0 < 512 else yhB
                            off = 0 if c0 < 512 else 512
                            nc.tensor.matmul(
                                half[:, c0 - off:c1 - off], vh[:, kc, :],
                                pt[:, c0:c1],
                                start=(kc == 0),
                                stop=(kc == (3 if half is yhA else NTC - 1)),
                                skip_group_check=True)
                        if kc == 3:
                            normalize_half(yhA, 0, pairi, bp, "A")
                    normalize_half(yhB, 1, pairi, bp, "B")

                # ---------- output projection ----------
                for tcn in range(NTC):
                    ostg = opool.tile([P, D], BF16, tag="ostg")
                    for oc in range(4):
                        ops_ = pss.tile([P, 512], F32, tag="stp",
                                        name=f"ops_{b}_{tcn}_{oc}")
                        for hc in range(2):
                            nc.tensor.matmul(
                                ops_[:], ytn[hc][:, tcn * P:(tcn + 1) * P],
                                t_wo[:, hc, oc * 512:(oc + 1) * 512],
                                start=(hc == 0), stop=(hc == 1))
                        if oc % 2 == 1:
                            nc.scalar.copy(
                                ostg[:, oc * 512:(oc + 1) * 512], ops_[:])
                        else:
                            nc.vector.tensor_copy(
                                ostg[:, oc * 512:(oc + 1) * 512], ops_[:])
                    nc.sync.dma_start(
                        out.ap()[tok0 + tcn * P: tok0 + (tcn + 1) * P, :],
                        ostg[:])

            # software pipeline: proj/rope of b+1 issued (higher priority)
            # before attention/outproj of b so PE always has filler work.
            prev = proj_rope_stage(0)
            for b in range(1, B):
                cur = proj_rope_stage(b)
                attn_out_stage(prev)
                prev = cur
            attn_out_stage(prev)

    nc.finalize()
    return nc


def _host_consts():
    inv = 1.0 / (ROPE_BASE ** (np.arange(0, HD, 2, dtype=np.float32) / HD))
    ang = np.arange(T, dtype=np.float32)[:, None] * inv[None, :]  # [T, 32]
    cosr = np.cos(ang).T.astype(np.float32)                        # [32, T]
    sinr = np.sin(ang).T.astype(np.float32)
    cc = np.tile(cosr, (4, 1))                                     # [128, T]
    ss = np.tile(np.concatenate([sinr, -sinr], axis=0), (2, 1))
    tri = np.triu(np.ones((P, P), np.float32))
    tri2 = np.stack([tri, tri], axis=1)                            # [128,2,128]
    return {
        "cc": np.ascontiguousarray(cc),
        "ss": np.ascontiguousarray(ss),
        "tri2": np.ascontiguousarray(tri2.astype(BF)),
        "o64": np.ones((64, 2), np.float32),
        "o1x64": np.ones((1, 64), np.float32),
        "idb": np.eye(64, dtype=np.float32).astype(BF),
    }


def kernel(x, Wq, Wk, Wv, Wo):
    x = np.asarray(x, np.float32)
    Wq = np.asarray(Wq, np.float32)
    Wk = np.asarray(Wk, np.float32)
    Wv = np.asarray(Wv, np.float32)
    Wo = np.asarray(Wo, np.float32)
    b, t, d = x.shape

    key = "nc"
    if key not in _COMPILED:
        _COMPILED[key] = _build_nc()
    nc = _COMPILED[key]

    xTh = np.ascontiguousarray(x.reshape(b * t, d).T.astype(BF))  # [2048, 4096]
    consts = _host_consts()

    in_maps = []
    for c in range(NCORES):
        wq_c = np.ascontiguousarray(
            Wq[:, c * DOUT:(c + 1) * DOUT].reshape(NDC, P, DOUT)
            .transpose(1, 0, 2).astype(BF))
        wkv_np = np.concatenate(
            [Wk[:, c * HD:(c + 1) * HD], Wv[:, c * HD:(c + 1) * HD]], axis=1)
        wkv_c = np.ascontiguousarray(
            wkv_np.reshape(NDC, P, 2 * HD).transpose(1, 0, 2).astype(BF))
        wo_c = np.ascontiguousarray(
            Wo[c * DOUT:(c + 1) * DOUT, :].reshape(2, P, d)
            .transpose(1, 0, 2).astype(BF))
        m = {"xT": xTh, "wq": wq_c, "wkv": wkv_c, "wo": wo_c}
        m.update(consts)
        in_maps.append(m)

    res = run_bass_kernel_spmd(nc, in_maps, list(range(NCORES)))
    acc = res.results[0]["out"].astype(np.float32)
    for c in range(1, NCORES):
        acc = acc + res.results[c]["out"].astype(np.float32)
    return acc.reshape(b, t, d)


if __name__ == "__main__":
    rng = np.random.default_rng(0)
    x = rng.standard_normal((B, T, D), dtype=np.float32)
    Wq = (rng.standard_normal((D, D), dtype=np.float32) * 0.02)
    Wk = (rng.standard_normal((D, KV), dtype=np.float32) * 0.02)
    Wv = (rng.standard_normal((D, KV), dtype=np.float32) * 0.02)
    Wo = (rng.standard_normal((D, D), dtype=np.float32) * 0.02)
    y = kernel(x=x, Wq=Wq, Wk=Wk, Wv=Wv, Wo=Wo)
    print("out", y.shape, y.dtype, np.abs(y).max())


# revision 6
# speedup vs baseline: 1.0014x; 1.0014x over previous
"""Trainium2 Bass kernel for ConformalGQA, v2.

Math identical to reference modulo softmax shift invariance: the -0.5|q|^2
term in the scores is constant over the softmax (key) axis, so it is dropped
entirely. Scores become (q.k - 0.5|k|^2)/8, bounded above by |q|^2/16 ~ 6, so
exp never overflows fp32 and needs no max pass. The -0.5|k|^2/8 term rides
the per-partition bias of the Exp activation.

Sharding: 8-way tensor-parallel over heads (core c: Q heads 4c..4c+3, KV
head c). Each core emits a full (4096, 2048) bf16 partial; host sums.

Per core, per batch (t=1024):
 - xT chunks DMA'd bf16; Wq/Wk/Wv column shards projected with weights
   stationary into PSUM f32 chunks [128, 512].
 - RoPE: PSUM chunk evicted to SBUF f32 (Act), cos-mul + signed-sin
   shifted-muls (shift = +-32 partitions; muls on GPSIMD, add on DVE),
   emitted as bf16 qhat/khat. khat duplicated to partitions 64:128 so both
   heads of a pair run S-matmuls via tile_position (0,0)/(64,0).
 - S^T computed per (head, kc) into [128, <=512] PSUM chunks with k on
   partitions (bf16 matmuls run 1 cyc/row at any free size); the diagonal
   block is zeroed post-exp by a triangular mask (DVE/Pool alternating).
 - P^T = Exp(S^T/8 + bias) -> bf16, bias = -0.0625|k|^2.
 - PV: yhat[65, q] += [V|1].T @ P^T accumulated over kc into two
   single-bank [65, 512] PSUM halves; row 64 is the softmax denominator.
   The low half completes at kc=3 and normalizes/frees early.
 - normalize: DVE reciprocal row, GPSIMD partition_broadcast, fused
   mul (PSUM x SBUF) -> ytn bf16.
 - out proj: ytn as lhsT against Wo row-shard, PSUM chunks evicted bf16
   (alternating DVE/Act) and DMA'd out per 128-token row block.
 - Whole thing software-pipelined: proj/rope of batch b+1 is issued at
   higher scheduler priority than attention/outproj of batch b.
 - NOTE: InstDmaTransposeAnt produced NaN on hardware under this runtime
   (CoreSim was clean) - V is transposed on the PE instead.
"""

import sys

for _p in ("/opt/trn_rl_repo",):
    if _p not in sys.path:
        sys.path.insert(0, _p)

import numpy as np
import ml_dtypes
from contextlib import ExitStack

import concourse.bass as bass
import concourse.mybir as mybir
import concourse.tile as tile
from concourse import bacc
from concourse.bass_utils import run_bass_kernel_spmd

F32R = mybir.dt.float32r
F32 = mybir.dt.float32
BF16 = mybir.dt.bfloat16
AF = mybir.ActivationFunctionType
BF = ml_dtypes.bfloat16

B, T, D, KV = 4, 1024, 2048, 512
H, HKV, HD = 32, 8, 64
P = 128
NCORES = 8
HPC = H // NCORES          # 4 q heads per core
DOUT = HPC * HD            # 256 q-proj cols per core
NDC = D // P               # 16 contraction chunks
NTC = T // P               # 8 token chunks per batch
ROPE_BASE = 10000.0

_COMPILED = {}


def _chunks_for(kc):
    """Natural S/PV q-chunks for key block kc (bf16: any width is full
    rate). Chunks never straddle the 512 boundary (PSUM half split)."""
    q0 = kc * P
    out = []
    c0 = q0
    while c0 < T:
        c1 = min(T, 512 if c0 < 512 else T)
        out.append((c0, c1))
        c0 = c1
    return q0, out


def _build_nc():
    nc = bacc.Bacc("TRN2", target_bir_lowering=False, debug=False,
                   num_devices=NCORES)

    xT = nc.dram_tensor("xT", [D, B * T], BF16, kind="ExternalInput")
    wq = nc.dram_tensor("wq", [P, NDC, DOUT], BF16, kind="ExternalInput")
    wkv = nc.dram_tensor("wkv", [P, NDC, 2 * HD], BF16, kind="ExternalInput")
    wo = nc.dram_tensor("wo", [P, 2, D], BF16, kind="ExternalInput")
    cc = nc.dram_tensor("cc", [P, T], F32, kind="ExternalInput")
    ss = nc.dram_tensor("ss", [P, T], F32, kind="ExternalInput")
    tri2 = nc.dram_tensor("tri2", [P, 2, P], BF16, kind="ExternalInput")
    o64 = nc.dram_tensor("o64", [64, 2], F32R, kind="ExternalInput")
    o1x64 = nc.dram_tensor("o1x64", [1, 64], F32R, kind="ExternalInput")
    idb = nc.dram_tensor("idb", [64, 64], BF16, kind="ExternalInput")
    out = nc.dram_tensor("out", [B * T, D], BF16, kind="ExternalOutput")

    with tile.TileContext(nc) as tc:
        with ExitStack() as ctx:
            cpool = ctx.enter_context(tc.tile_pool(name="consts", bufs=1))
            wpool = ctx.enter_context(tc.tile_pool(name="weights", bufs=1))
            xpool = ctx.enter_context(tc.tile_pool(name="x", bufs=8))
            spool = ctx.enter_context(tc.tile_pool(name="stage", bufs=4))
            qpool = ctx.enter_context(tc.tile_pool(name="qk", bufs=2))
            vpool = ctx.enter_context(tc.tile_pool(name="v", bufs=2))
            fpool = ctx.enter_context(tc.tile_pool(name="pt", bufs=3))
            npool = ctx.enter_context(tc.tile_pool(name="norm", bufs=4))
            ypool = ctx.enter_context(tc.tile_pool(name="ytn", bufs=2))
            opool = ctx.enter_context(tc.tile_pool(name="ostg", bufs=3))
            psy = ctx.enter_context(tc.tile_pool(name="psy", bufs=2, space="PSUM"))
            pss = ctx.enter_context(tc.tile_pool(name="pss", bufs=4, space="PSUM"))
            psm = ctx.enter_context(tc.tile_pool(name="psm", bufs=2, space="PSUM"))

            # ---- early consts (needed by first projections/rope) ----
            t_wkv = wpool.tile([P, NDC, 2 * HD], BF16)
            nc.sync.dma_start(t_wkv[:], wkv.ap())
            t_wq = wpool.tile([P, NDC, DOUT], BF16)
            t_cc = cpool.tile([P, T], F32)
            t_ss = cpool.tile([P, T], F32)

            xT3 = xT.ap().rearrange("(c p) t -> p c t", p=P)  # [128, 16, 4096]

            def late_consts():
                t_tri2 = cpool.tile([P, 2, P], BF16)
                nc.sync.dma_start(t_tri2[:], tri2.ap())
                t_o64 = cpool.tile([64, 2], F32R)
                nc.sync.dma_start(t_o64[:], o64.ap())
                t_o1x64 = cpool.tile([1, 64], F32R)
                nc.sync.dma_start(t_o1x64[:], o1x64.ap())
                t_idb = cpool.tile([64, 64], BF16)
                nc.sync.dma_start(t_idb[:], idb.ap())
                t_wo = wpool.tile([P, 2, D], BF16)
                nc.sync.dma_start(t_wo[:], wo.ap())
                return t_tri2, t_o64, t_wo, t_o1x64, t_idb

            lc = None

            def rope_half(pj, dst, rows, half, sign_dup):
                """Evict PSUM proj chunk, rope it, write bf16 into dst."""
                c0 = half * 512
                sb = spool.tile([P, 512], F32, tag="qsb")
                nc.vector.tensor_copy(sb[0:rows, :], pj[0:rows, :])
                t1 = spool.tile([P, 512], F32, tag="t1")
                nc.vector.tensor_mul(
                    t1[0:rows, :], sb[0:rows, :], t_cc[0:rows, c0:c0 + 512])
                t2 = spool.tile([P, 512], F32, tag="t2")
                for bp2 in range(0, rows, 64):
                    nc.gpsimd.tensor_mul(
                        t2[bp2:bp2 + 32, :], sb[bp2 + 32:bp2 + 64, :],
                        t_ss[bp2 + 32:bp2 + 64, c0:c0 + 512])
                    nc.gpsimd.tensor_mul(
                        t2[bp2 + 32:bp2 + 64, :], sb[bp2:bp2 + 32, :],
                        t_ss[bp2:bp2 + 32, c0:c0 + 512])
                nc.vector.tensor_add(
                    dst[0:rows, c0:c0 + 512], t1[0:rows, :], t2[0:rows, :])
                if sign_dup:
                    nc.vector.tensor_copy(
                        dst[64:128, c0:c0 + 512], dst[0:64, c0:c0 + 512])

            def proj_rope_stage(b):
                """Load xT for batch b, project Q/K/V, rope, prep vh/kb."""
                nonlocal lc
                tok0 = b * T
                xts = []
                for qtr in range(4):
                    xt = xpool.tile([P, 4, T], BF16, tag="xt",
                                    name=f"xt_{b}_{qtr}")
                    if b == 0:
                        # fine-grained loads so batch-0 projections start
                        # as soon as the first contraction chunk lands;
                        # wq arrives in pair halves, rope tables last
                        for i in range(4):
                            nc.sync.dma_start(
                                xt[:, i, :],
                                xT3[:, qtr * 4 + i, tok0:tok0 + T])
                            if qtr == 0 and i == 0:
                                nc.sync.dma_start(
                                    t_wq[:, :, 0:P], wq.ap()[:, :, 0:P])
                        if qtr == 1:
                            nc.sync.dma_start(
                                t_wq[:, :, P:2 * P], wq.ap()[:, :, P:2 * P])
                        if qtr == 3:
                            nc.sync.dma_start(t_cc[:], cc.ap())
                            nc.sync.dma_start(t_ss[:], ss.ap())
                    else:
                        nc.sync.dma_start(
                            xt[:], xT3[:, qtr * 4:(qtr + 1) * 4, tok0:tok0 + T])
                    xts.append(xt)
                if b == 0:
                    lc = late_consts()

                def xsrc(dc):
                    return xts[dc // 4][:, dc % 4, :]

                qh = [qpool.tile([P, T], BF16, tag="qh", bufs=4,
                                 name=f"qh_{b}_{i}") for i in range(2)]
                kh = qpool.tile([P, T], BF16, tag="kh", name=f"kh_{b}")
                k2 = qpool.tile([64, T], F32R, tag="k2", name=f"k2_{b}")
                vt = vpool.tile([64, T], BF16, tag="vt", name=f"vt_{b}")

                # interleave kv and q-pair0 chunks so both pj slots
                # stream against arriving xT chunks; then q-pair1.
                def kv_chunk(half):
                    pj = psm.tile([P, 512], F32, tag="pj",
                                  name=f"kvpj_{b}_{half}")
                    for dc in range(NDC):
                        nc.tensor.matmul(
                            pj[:], t_wkv[:, dc, :],
                            xsrc(dc)[:, half * 512:(half + 1) * 512],
                            start=(dc == 0), stop=(dc == NDC - 1))
                    # kv proj out rows 0:64 = K dims, 64:128 = V dims.
                    nc.vector.tensor_copy(vt[:, half * 512:(half + 1) * 512],
                                          pj[64:128, :])
                    rope_half(pj, kh, 64, half, sign_dup=True)

                def q_chunk(pairi, half):
                    pj = psm.tile([P, 512], F32, tag="pj",
                                  name=f"qpj_{b}_{pairi}_{half}")
                    for dc in range(NDC):
                        nc.tensor.matmul(
                            pj[:],
                            t_wq[:, dc, pairi * P:(pairi + 1) * P],
                            xsrc(dc)[:, half * 512:(half + 1) * 512],
                            start=(dc == 0), stop=(dc == NDC - 1))
                    rope_half(pj, qh[pairi], 128, half, sign_dup=False)

                kv_chunk(0)
                q_chunk(0, 0)
                kv_chunk(1)
                q_chunk(0, 1)
                q_chunk(1, 0)
                q_chunk(1, 1)

                # |k|^2 -> per-partition bias  (transposed via PE)
                t_o64, t_idb = lc[1], lc[4]
                nc.scalar.activation(k2[:], kh[0:64, :], AF.Square)
                nsq = psm.tile([P, 512], F32, tag="pj", name=f"nsq_{b}")
                for kc in range(NTC):
                    nc.tensor.matmul(
                        nsq[:, 2 * kc:2 * kc + 2], k2[:, kc * P:(kc + 1) * P],
                        t_o64[:], start=True, stop=True)
                kb = qpool.tile([P, NTC], F32, tag="kb", name=f"kb_{b}")
                nc.vector.tensor_scalar_mul(
                    kb[:],
                    nsq[:, 0:2 * NTC]
                    .rearrange("p (c two) -> p c two", two=2)[:, :, 0],
                    -0.0625)

                # V transposed into [token, hd | 1] layout via PE transpose
                vh = vpool.tile([P, NTC, HD + 1], BF16, tag="vh",
                                name=f"vh_{b}")
                nc.vector.memset(vh[:, :, HD:HD + 1], 1.0)
                for tcn in range(NTC):
                    tp = pss.tile([P, 64], BF16, tag="stp", name=f"tp_{b}_{tcn}")
                    nc.tensor.transpose(
                        tp[:], vt[:, tcn * P:(tcn + 1) * P], t_idb[:])
                    nc.scalar.copy(vh[:, tcn, 0:HD], tp[:])
                return dict(b=b, qh=qh, kh=kh, kb=kb, vh=vh)

            def attn_out_stage(st):
                b, qh, kh, kb, vh = st["b"], st["qh"], st["kh"], st["kb"], st["vh"]
                tok0 = b * T
                t_tri2, t_o64, t_wo, t_o1x64, t_idb = lc
                ytn = [ypool.tile([P, T], BF16, tag="ytn", bufs=4,
                                  name=f"ytn_{b}_{i}") for i in range(2)]
                def normalize_half(yh_half, hq, pairi, bp, tag):
                    """One half of softmax-normalize as soon as its PV
                    contributions are complete; frees the yh slot early."""
                    rsb = npool.tile([1, 512], F32R, tag="rsb")
                    with nc.allow_low_precision(reason="recip row"):
                        nc.vector.reciprocal(rsb[:], yh_half[64:65, :])
                    rbc = npool.tile([64, 512], F32R, tag="rbc")
                    nc.gpsimd.partition_broadcast(rbc[:], rsb[:])
                    nc.vector.tensor_mul(
                        ytn[pairi][bp:bp + 64, hq * 512:(hq + 1) * 512],
                        yh_half[0:64, :], rbc[:])

                for h in range(HPC):
                    pairi, bp = h // 2, 64 * (h % 2)
                    yhA = psy.tile([65, 512], F32, tag="yh",
                                   name=f"yhA_{b}_{h}")
                    yhB = psy.tile([65, 512], F32, tag="yh",
                                   name=f"yhB_{b}_{h}")
                    for kc in range(NTC):
                        q0 = kc * P
                        _, chs = _chunks_for(kc)
                        pt = fpool.tile([P, T], BF16, tag="pt", bufs=6)
                        for (c0, c1) in chs:
                            stp = pss.tile([P, 512], F32, tag="stp")
                            nc.tensor.matmul(
                                stp[:, 0:c1 - c0],
                                kh[bp:bp + 64, kc * P:(kc + 1) * P],
                                qh[pairi][bp:bp + 64, c0:c1],
                                start=True, stop=True,
                                tile_position=(bp, 0))
                            nc.scalar.activation(
                                pt[:, c0:c1], stp[:, 0:c1 - c0], AF.Exp,
                                bias=kb[:, kc:kc + 1], scale=0.125)
                        # causal mask on the diagonal block
                        meng = nc.vector if kc % 2 == 0 else nc.gpsimd
                        meng.tensor_mul(
                            pt[:, q0:q0 + P], pt[:, q0:q0 + P],
                            t_tri2[:, 0, :])
                        for (c0, c1) in chs:
                            half = yhA if c0 < 512 else yhB
                            off = 0 if c0 < 512 else 512
                            nc.tensor.matmul(
                                half[:, c0 - off:c1 - off], vh[:, kc, :],
                                pt[:, c0:c1],
                                start=(kc == 0),
                                stop=(kc == (3 if half is yhA else NTC - 1)),
                                skip_group_check=True)
                        if kc == 3:
                            normalize_half(yhA, 0, pairi, bp, "A")
                    normalize_half(yhB, 1, pairi, bp, "B")

                # ---------- output projection ----------
                for tcn in range(NTC):
                    ostg = opool.tile([P, D], BF16, tag="ostg")
                    for oc in range(4):
                        ops_ = pss.tile([P, 512], F32, tag="stp",
                                        name=f"ops_{b}_{tcn}_{oc}")
                        for hc in range(2):
                            nc.tensor.matmul(
                                ops_[:], ytn[hc][:, tcn * P:(tcn + 1) * P],
                                t_wo[:, hc, oc * 512:(oc + 1) * 512],
                                start=(hc == 0), stop=(hc == 1))
                        if oc % 2 == 1:
                            nc.scalar.copy(
                                ostg[:, oc * 512:(oc + 1) * 512], ops_[:])
                        else:
                            nc.vector.tensor_copy(
                                ostg[:, oc * 512:(oc + 1) * 512], ops_[:])
                    nc.sync.dma_start(
                        out.ap()[tok0 + tcn * P: tok0 + (tcn + 1) * P, :],
                        ostg[:])

            # software pipeline: proj/rope of b+1 issued (higher priority)
            # before attention/outproj of b so PE always has filler work.
            prev = proj_rope_stage(0)
            for b in range(1, B):
                cur = proj_rope_stage(b)
                attn_out_stage(prev)
                prev = cur
            attn_out_stage(prev)

    nc.finalize()
    return nc


def _host_consts():
    inv = 1.0 / (ROPE_BASE ** (np.arange(0, HD, 2, dtype=np.float32) / HD))
    ang = np.arange(T, dtype=np.float32)[:, None] * inv[None, :]  # [T, 32]
    cosr = np.cos(ang).T.astype(np.float32)                        # [32, T]
    sinr = np.sin(ang).T.astype(np.float32)
    cc = np.tile(cosr, (4, 1))                                     # [128, T]
    ss = np.tile(np.concatenate([sinr, -sinr], axis=0), (2, 1))
    tri = np.triu(np.ones((P, P), np.float32))
    tri2 = np.stack([tri, tri], axis=1)                            # [128,2,128]
    return {
        "cc": np.ascontiguousarray(cc),
        "ss": np.ascontiguousarray(ss),
        "tri2": np.ascontiguousarray(tri2.astype(BF)),
        "o64": np.ones((64, 2), np.float32),
        "o1x64": np.ones((1, 64), np.float32),
        "idb": np.eye(64, dtype=np.float32).astype(BF),
    }


def kernel(x, Wq, Wk, Wv, Wo):
    x = np.asarray(x, np.float32)
    Wq = np.asarray(Wq, np.float32)
    Wk = np.asarray(Wk, np.float32)
    Wv = np.asarray(Wv, np.float32)
    Wo = np.asarray(Wo, np.float32)
    b, t, d = x.shape

    key = "nc"
    if key not in _COMPILED:
        _COMPILED[key] = _build_nc()
    nc = _COMPILED[key]

    xTh = np.ascontiguousarray(x.reshape(b * t, d).T.astype(BF))  # [2048, 4096]
    consts = _host_consts()

    in_maps = []
    for c in range(NCORES):
        wq_c = np.ascontiguousarray(
            Wq[:, c * DOUT:(c + 1) * DOUT].reshape(NDC, P, DOUT)
            .transpose(1, 0, 2).astype(BF))
        wkv_np = np.concatenate(
            [Wk[:, c * HD:(c + 1) * HD], Wv[:, c * HD:(c + 1) * HD]], axis=1)
        wkv_c = np.ascontiguousarray(
            wkv_np.reshape(NDC, P, 2 * HD).transpose(1, 0, 2).astype(BF))
        wo_c = np.ascontiguousarray(
            Wo[c * DOUT:(c + 1) * DOUT, :].reshape(2, P, d)
            .transpose(1, 0, 2).astype(BF))
        m = {"xT": xTh, "wq": wq_c, "wkv": wkv_c, "wo": wo_c}
        m.update(consts)
        in_maps.append(m)

    res = run_bass_kernel_spmd(nc, in_maps, list(range(NCORES)))
    acc = res.results[0]["out"].astype(np.float32)
    for c in range(1, NCORES):
        acc = acc + res.results[c]["out"].astype(np.float32)
    return acc.reshape(b, t, d)


if __name__ == "__main__":
    rng = np.random.default_rng(0)
    x = rng.standard_normal((B, T, D), dtype=np.float32)
    Wq = (rng.standard_normal((D, D), dtype=np.float32) * 0.02)
    Wk = (rng.standard_normal((D, KV), dtype=np.float32) * 0.02)
    Wv = (rng.standard_normal((D, KV), dtype=np.float32) * 0.02)
    Wo = (rng.standard_normal((D, D), dtype=np.float32) * 0.02)
    y = kernel(x=x, Wq=Wq, Wk=Wk, Wv=Wv, Wo=Wo)
    print("out", y.shape, y.dtype, np.abs(y).max())
